# revision 1
# baseline (speedup 1.0000x reference)
"""ChebConvolution (K=4) Trainium2 kernel, 8-way sharded.

Math: with P = spmm(2*adj_vals) and right-multiplication by W commuting
with the (linear) sparse propagation, the reference collapses to

    Y = P(P X) W^3 - (P X) W^3 - X W^2
      = P(table2) - V,   table2 = Z1 @ W^3,  Z1 = P X,
                         V = Z1 @ W^3 + X @ W^2

Per core c (rows [c*S, (c+1)*S)):
  phase 1: SpMM Z1 rows via dma_gather from replicated bf16 X table +
           one-hot*val mask matmuls accumulated in PSUM (feature-major),
           then t2 = Z1 @ W3 and V = t2 + X @ W2 per 128-row block.
  AllGather t2 shards -> full bf16 table2 in every core's HBM.
  phase 2: SpMM P(table2) rows (node-major psum), subtract V, write Y shard.

The mask trick: for a tile of 128 edges, M[e, d] = (IOTA[e,d] == dest[e]) *
val[e] is ONE DVE tensor_scalar (is_equal, mult) with per-partition scalars;
psum += M.T @ G (or G.T @ M) does scale + segment-sum on the PE. Edges are
sorted by dest; a 128-row dest block maps to a (cross-core union) range of
tiles, and boundary tiles are simply multiplied into both adjacent blocks'
psums - out-of-block dests never match the IOTA so they contribute zero.
This avoids all per-block gather padding: Q7 SWDGE descriptor generation
(~8ns/idx) is the bottleneck, so gathered-edge count is minimized while the
(cheap) DVE mask builds absorb the schedule overlap.

Edges are partitioned by dest core and split by source half (int16 gather
index limit); per-(core,half) streams are padded only at the end to a
cross-core-uniform tile count so one NEFF serves all 8 cores.
"""

import os
import sys

for _p in ("/opt/trn_rl_repo", "/root/.axon_site/_ro/trn_rl_repo"):
    if os.path.isdir(_p) and _p not in sys.path:
        sys.path.insert(0, _p)

import numpy as np
import ml_dtypes

import concourse.bacc as bacc
import concourse.mybir as mybir
import concourse.tile as tile
from concourse.bass_utils import run_bass_kernel_spmd

F32 = mybir.dt.float32
BF16 = mybir.dt.bfloat16
I16 = mybir.dt.int16

D = 128            # feature dim (in == out == 128)
SPLIT = 32768      # int16 gather index limit -> lo/hi table halves
CH_TILES = 8       # gather chunk: 1024 idx (HW dma_gather limit)
PAD_DEST = 3.0e8   # dest sentinel for padding edges (never matches IOTA)


def _pack_idxs(flat_idx):
    """int16 gather index layout: [128, n/16], idx j at [16k + j%16, j//16]."""
    n = len(flat_idx)
    assert n % 16 == 0
    arr = flat_idx.astype(np.int16).reshape(n // 16, 16).T  # [16, n/16]
    return np.tile(arr, (8, 1))


def _host_prep(N, ncores, adj_rows, adj_cols, adj_vals):
    """Sort/pad edges into per-core uniform tile streams + union schedule.

    Returns sched (cross-core constants incl. per-block mm lists) and
    per-core input arrays.
    """
    S = N // ncores
    NB = (S + 127) // 128
    rows = adj_rows.astype(np.int64)
    cols = adj_cols.astype(np.int64)
    vals2 = (2.0 * adj_vals).astype(np.float32)

    core = rows // S
    dloc = rows - core * S
    half = (cols >= SPLIT).astype(np.int64)

    # per-(core, half) edge counts -> uniform padded tile counts
    ch_key = core * 2 + half
    cnt = np.bincount(ch_key, minlength=ncores * 2).reshape(ncores, 2)
    T_half = [max(int(-(-cnt[:, h].max() // 128)), 1) for h in (0, 1)]
    if N <= SPLIT:
        T_half[1] = 0

    # order edges by (core, half, dest, col); rank within (core, half)
    order = np.lexsort((cols, dloc, half, core))
    k_s = ch_key[order]
    firsts = np.r_[0, np.flatnonzero(np.diff(k_s)) + 1]
    seg_of = np.cumsum(np.isin(np.arange(len(k_s)), firsts)) - 1
    rank = np.arange(len(k_s)) - firsts[seg_of]

    core_s, half_s = core[order], half[order]
    col_s, dloc_s, val_s = cols[order], dloc[order], vals2[order]
    tile_in_half = rank // 128

    # Per (half, tile): anchor block (global min dest across cores) and
    # straddle flag. Masks are 256 wide, so every tile's dests must fit in
    # [128*anchor, 128*anchor + 256) across ALL cores.
    anchors, straddles = [], []
    for h in range(2):
        T = max(T_half[h], 1)
        tmin = np.full(T, 1 << 30, np.int64)
        tmax = np.full(T, -1, np.int64)
        m = half_s == h
        if m.any():
            np.minimum.at(tmin, tile_in_half[m], dloc_s[m])
            np.maximum.at(tmax, tile_in_half[m], dloc_s[m])
        anchor = np.where(tmax >= 0, np.minimum(tmin // 128, NB - 1), 0)
        assert (tmax < anchor * 128 + 384).all(), "tile dest span exceeds 384"
        anchors.append(anchor)
        straddles.append((tmax >= (anchor + 1) * 128,
                          tmax >= (anchor + 2) * 128))

    # per-block mm list: (half, tile, slice); slice 2 uses the f32 iota2
    block_mms = [[] for _ in range(NB)]
    for h in range(2):
        if T_half[h] == 0:
            continue
        for t in range(T_half[h]):
            b = int(anchors[h][t])
            block_mms[b].append((h, t, 0))
            if straddles[h][0][t]:
                block_mms[b + 1].append((h, t, 1))
            if straddles[h][1][t]:
                block_mms[b + 2].append((h, t, 2))
    assert all(block_mms[b] for b in range(NB)), "block with no matmuls"
    NMM = sum(len(m) for m in block_mms)

    # fill per-core data arrays (dest/val are per-TILE columns; lo then hi)
    T_tot = T_half[0] + T_half[1]
    idx_streams = [np.zeros((ncores, max(T_half[h], 1) * 128), np.int64)
                   for h in range(2)]
    dest_all = np.full((ncores, T_tot * 128), PAD_DEST, np.float32)
    val_all = np.zeros((ncores, T_tot * 128), np.float32)

    slot = rank
    for h in range(2):
        m = half_s == h
        if not m.any():
            continue
        idx_streams[h][core_s[m], slot[m]] = col_s[m] - (SPLIT if h else 0)

    tcol = tile_in_half + np.where(half_s == 1, T_half[0], 0)
    t0c = np.minimum(tile_in_half, max(T_half[0], 1) - 1)
    t1c = np.minimum(tile_in_half, max(T_half[1], 1) - 1)
    anchor_s = np.where(half_s == 0, anchors[0][t0c], anchors[1][t1c])
    rel = dloc_s - 128 * anchor_s
    assert ((rel >= 0) & (rel < 384)).all()
    gpos = core_s * (T_tot * 128) + tcol * 128 + (slot % 128)
    dest_all.reshape(-1)[gpos] = rel.astype(np.float32)
    val_all.reshape(-1)[gpos] = val_s

    sched = dict(S=S, NB=NB, T_lo=T_half[0], T_hi=T_half[1], NMM=NMM,
                 T_tot=T_tot, block_mms=block_mms)
    per_core = []
    for c in range(ncores):
        per_core.append(dict(
            idx_lo=_pack_idxs(idx_streams[0][c]),
            idx_hi=_pack_idxs(idx_streams[1][c]) if T_half[1] else None,
            dest=np.ascontiguousarray(
                dest_all[c].reshape(T_tot, 128).T),
            val=np.ascontiguousarray(val_all[c].reshape(T_tot, 128).T),
        ))
    return sched, per_core


ACT_EVERY = 4  # every ACT_EVERY-th mask build goes to the Scalar engine


def _build_program(N, ncores, sched):
    S, NB = sched["S"], sched["NB"]
    T_lo, T_hi, T_tot = sched["T_lo"], sched["T_hi"], sched["T_tot"]
    block_mms = sched["block_mms"]

    nc = bacc.Bacc("TRN2", target_bir_lowering=False,
                   num_devices=(ncores if ncores > 1 else None))

    tab_d = nc.dram_tensor("tab", [N, D], BF16, kind="ExternalInput")
    xT_d = nc.dram_tensor("xT", [D, NB * 128], BF16, kind="ExternalInput")
    w_d = nc.dram_tensor("w", [D, D], F32, kind="ExternalInput")
    wT_d = nc.dram_tensor("wT", [D, D], F32, kind="ExternalInput")
    iota_d = nc.dram_tensor("iota", [128, 256], BF16, kind="ExternalInput")
    iota2_d = nc.dram_tensor("iota2", [128, 128], F32, kind="ExternalInput")
    ixlo_d = nc.dram_tensor("ixlo", [128, T_lo * 8], I16, kind="ExternalInput")
    if T_hi:
        ixhi_d = nc.dram_tensor("ixhi", [128, T_hi * 8], I16, kind="ExternalInput")
    dest_d = nc.dram_tensor("dest", [128, T_tot], F32, kind="ExternalInput")
    val_d = nc.dram_tensor("val", [128, T_tot], F32, kind="ExternalInput")
    y_d = nc.dram_tensor("y", [S, D], F32, kind="ExternalOutput")

    cc_in = nc.dram_tensor("cc_in", [S, D], BF16, kind="Internal")
    cc_out = nc.dram_tensor("cc_out", [N, D], BF16, kind="Internal",
                            addr_space="Shared")

    ixlo_sb = nc.alloc_sbuf_tensor("ixlo_sb", [128, T_lo * 8], I16)
    ixhi_sb = nc.alloc_sbuf_tensor("ixhi_sb", [128, T_hi * 8], I16) if T_hi else None
    dest_sb = nc.alloc_sbuf_tensor("dest_sb", [128, T_tot], F32)
    val_sb = nc.alloc_sbuf_tensor("val_sb", [128, T_tot], F32)
    nval_sb = nc.alloc_sbuf_tensor("nval_sb", [128, T_tot], F32)
    iota_sb = nc.alloc_sbuf_tensor("iota_sb", [128, 256], BF16)
    iota2_sb = nc.alloc_sbuf_tensor("iota2_sb", [128, 128], F32)
    xT_sb = nc.alloc_sbuf_tensor("xT_sb", [D, NB * 128], BF16)
    w_sb = nc.alloc_sbuf_tensor("w_sb", [D, D], F32)
    wT_sb = nc.alloc_sbuf_tensor("wT_sb", [D, D], F32)
    w2_sb = nc.alloc_sbuf_tensor("w2_sb", [D, D], F32)
    w2bf_sb = nc.alloc_sbuf_tensor("w2bf_sb", [D, D], BF16)
    w3bf_sb = nc.alloc_sbuf_tensor("w3bf_sb", [D, D], BF16)
    v_sb = nc.alloc_sbuf_tensor("v_sb", [128, NB * 128], F32)

    n_tiles = (T_lo, T_hi)

    def chunks(tot):
        out = []
        t0 = 0
        while t0 < tot:
            ct = min(CH_TILES, tot - t0)
            out.append((t0, ct))
            t0 += ct
        return out

    stream_chunks = (chunks(T_lo), chunks(T_hi))

    with tile.TileContext(nc) as tc:
        nc.sync.dma_start(ixlo_sb[:], ixlo_d[:])
        if T_hi:
            nc.sync.dma_start(ixhi_sb[:], ixhi_d[:])
        nc.sync.dma_start(dest_sb[:], dest_d[:])
        nc.sync.dma_start(val_sb[:], val_d[:])
        nc.vector.tensor_scalar(nval_sb[:], val_sb[:], -1.0, None,
                                mybir.AluOpType.mult)
        nc.sync.dma_start(iota_sb[:], iota_d[:])
        nc.sync.dma_start(iota2_sb[:], iota2_d[:])
        nc.sync.dma_start(xT_sb[:], xT_d[:])
        nc.sync.dma_start(w_sb[:], w_d[:])
        nc.sync.dma_start(wT_sb[:], wT_d[:])

        with (
            tc.tile_pool(name="wps", bufs=2, space="PSUM") as wps,
            tc.tile_pool(name="wsb", bufs=2) as wsb,
        ):
            w2_ps = wps.tile([D, D], F32, name="w2_ps")
            nc.tensor.matmul(w2_ps[:], wT_sb[:], w_sb[:], start=True, stop=True)
            nc.vector.tensor_copy(w2_sb[:], w2_ps[:])
            nc.vector.tensor_copy(w2bf_sb[:], w2_ps[:])
            w3_ps = wps.tile([D, D], F32, name="w3_ps")
            nc.tensor.matmul(w3_ps[:], wT_sb[:], w2_sb[:], start=True, stop=True)
            nc.vector.tensor_copy(w3bf_sb[:], w3_ps[:])

        def emit_spmm(phase, tab_lo_ap, tab_hi_ap, per_block_tail):
            with (
                tc.tile_pool(name=f"g{phase}", bufs=3) as gpool,
                tc.tile_pool(name=f"m{phase}", bufs=24) as mpool,
                tc.tile_pool(name=f"a{phase}", bufs=6) as apool,
                tc.tile_pool(name=f"ps{phase}", bufs=3, space="PSUM") as ppool,
                tc.tile_pool(name=f"tail{phase}", bufs=2, space="PSUM") as tpool,
                tc.tile_pool(name=f"sb{phase}", bufs=3) as spool,
            ):
                gbufs = {}
                masks = {}
                masks2 = {}
                nmask = [0]

                def ensure_chunk(h, ci):
                    k = (h, ci)
                    if k in gbufs:
                        return gbufs[k]
                    t0, ct = stream_chunks[h][ci]
                    n = ct * 128
                    g = gpool.tile([128, CH_TILES, 128], BF16,
                                   tag=f"g{h}", name=f"g{phase}_{h}_{ci}")
                    ix = (ixlo_sb, ixhi_sb)[h]
                    tab = (tab_lo_ap, tab_hi_ap)[h]
                    nc.gpsimd.dma_gather(
                        g[:, :ct, :], tab, ix[:, t0 * 8:(t0 + ct) * 8], n, n, D)
                    gbufs[k] = g
                    return g

                def ensure_mask(h, t):
                    k = (h, t)
                    if k in masks:
                        return masks[k]
                    i = t + (T_lo if h else 0)
                    m = mpool.tile([128, 256], BF16, tag="m",
                                   name=f"m{phase}_{h}_{t}")
                    if nmask[0] % ACT_EVERY == ACT_EVERY - 1:
                        # ACT build: a1=|iota-dest|; m=Relu(val - val*a1)
                        a1 = apool.tile([128, 256], F32, tag="a1",
                                        name=f"a1_{phase}_{h}_{t}")
                        nc.scalar.activation(
                            a1[:], iota_sb[:],
                            mybir.ActivationFunctionType.Abs,
                            bias=dest_sb[:, i:i + 1], scale=-1.0)
                        nc.scalar.activation(
                            m[:], a1[:],
                            mybir.ActivationFunctionType.Relu,
                            bias=val_sb[:, i:i + 1],
                            scale=nval_sb[:, i:i + 1])
                    else:
                        nc.vector.tensor_scalar(
                            m[:], iota_sb[:],
                            dest_sb[:, i:i + 1], val_sb[:, i:i + 1],
                            mybir.AluOpType.is_equal, mybir.AluOpType.mult)
                    nmask[0] += 1
                    masks[k] = m
                    return m

                def ensure_mask2(h, t):
                    k = (h, t)
                    if k in masks2:
                        return masks2[k]
                    i = t + (T_lo if h else 0)
                    m2 = mpool.tile([128, 128], BF16, tag="m2",
                                    name=f"m2_{phase}_{h}_{t}")
                    nc.vector.tensor_scalar(
                        m2[:], iota2_sb[:],
                        dest_sb[:, i:i + 1], val_sb[:, i:i + 1],
                        mybir.AluOpType.is_equal, mybir.AluOpType.mult)
                    masks2[k] = m2
                    return m2

                for b in range(NB):
                    mms = block_mms[b]
                    ps = ppool.tile([128, 128], F32, tag="ps", name=f"ps{phase}_{b}")
                    for j, (h, t, sl) in enumerate(mms):
                        g = ensure_chunk(h, t // CH_TILES)
                        tic = t % CH_TILES
                        if sl == 2:
                            msl = ensure_mask2(h, t)[:, :]
                        else:
                            m = ensure_mask(h, t)
                            msl = m[:, sl * 128:(sl + 1) * 128]
                        first, last = (j == 0), (j == len(mms) - 1)
                        if phase == 1:
                            nc.tensor.matmul(ps[:], g[:, tic, :], msl,
                                             start=first, stop=last)
                        else:
                            nc.tensor.matmul(ps[:], msl, g[:, tic, :],
                                             start=first, stop=last)
                    per_block_tail(b, ps, tpool, spool)

        def tail1(b, ps, tpool, spool):
            rows = min(128, S - 128 * b)
            z1t = spool.tile([128, 128], BF16, tag="z1t", name=f"z1t_{b}")
            nc.scalar.copy(z1t[:], ps[:])                      # ACT [f,d] bf16
            t2_ps = tpool.tile([128, 128], F32, tag="t2ps", name=f"t2ps_{b}")
            nc.tensor.matmul(t2_ps[:], z1t[:], w3bf_sb[:], start=True, stop=True)
            u_ps = tpool.tile([128, 128], F32, tag="ups", name=f"ups_{b}")
            nc.tensor.matmul(u_ps[:], xT_sb[:, b * 128:(b + 1) * 128],
                             w2bf_sb[:], start=True, stop=True)
            t2t = spool.tile([128, 128], BF16, tag="t2t", name=f"t2t_{b}")
            nc.scalar.copy(t2t[:], t2_ps[:])                   # ACT f32->bf16
            nc.vector.tensor_tensor(v_sb[:, b * 128:(b + 1) * 128],
                                    u_ps[:], t2t[:], mybir.AluOpType.add)
            nc.sync.dma_start(cc_in[b * 128:b * 128 + rows, :], t2t[:rows, :])

        def tail2(b, ps, tpool, spool):
            rows = min(128, S - 128 * b)
            y = spool.tile([128, 128], F32, tag="y", name=f"y_{b}")
            nc.vector.tensor_tensor(y[:], ps[:], v_sb[:, b * 128:(b + 1) * 128],
                                    mybir.AluOpType.subtract)
            nc.sync.dma_start(y_d[b * 128:b * 128 + rows, :], y[:rows, :])

        hi_rows = N - SPLIT if N > SPLIT else 0
        emit_spmm(1, tab_d[0:min(SPLIT, N), :],
                  tab_d[SPLIT:N, :] if hi_rows else None, tail1)

        if ncores > 1:
            nc.gpsimd.collective_compute(
                "AllGather", mybir.AluOpType.bypass,
                replica_groups=[list(range(ncores))],
                ins=[cc_in[:]], outs=[cc_out[:]])
        else:
            nc.sync.dma_start(cc_out[:], cc_in[:])

        emit_spmm(2, cc_out[0:min(SPLIT, N), :],
                  cc_out[SPLIT:N, :] if hi_rows else None, tail2)

    nc.compile()
    return nc


def _make_in_maps(N, ncores, sched, per_core, input_np, W_np):
    S, NB = sched["S"], sched["NB"]
    tab = input_np.astype(ml_dtypes.bfloat16)
    iota = np.broadcast_to(np.arange(256, dtype=np.float32),
                           (128, 256)).astype(ml_dtypes.bfloat16).copy()
    iota2 = np.ascontiguousarray(np.broadcast_to(
        np.arange(256, 384, dtype=np.float32), (128, 128)))
    W = W_np.astype(np.float32)
    WT = np.ascontiguousarray(W.T)
    in_maps = []
    for c in range(ncores):
        xT = np.zeros((D, NB * 128), ml_dtypes.bfloat16)
        xT[:, :S] = tab[c * S:(c + 1) * S].T
        m = dict(tab=tab, xT=xT, w=W, wT=WT, iota=iota, iota2=iota2,
                 ixlo=per_core[c]["idx_lo"],
                 dest=per_core[c]["dest"], val=per_core[c]["val"])
        if sched["T_hi"]:
            m["ixhi"] = per_core[c]["idx_hi"]
        in_maps.append(m)
    return in_maps


_cache = {}


def _get_program(N, ncores, sched):
    key = (N, ncores, sched["NMM"], sched["T_lo"], sched["T_hi"])
    if key not in _cache:
        _cache[key] = _build_program(N, ncores, sched)
    return _cache[key]


def run(input, adj_rows, adj_cols, adj_vals, W, ncores=8, trace=False):
    N = input.shape[0]
    sched, per_core = _host_prep(N, ncores, adj_rows, adj_cols, adj_vals)
    nc = _get_program(N, ncores, sched)
    in_maps = _make_in_maps(N, ncores, sched, per_core, np.asarray(input),
                            np.asarray(W))
    res = run_bass_kernel_spmd(nc, in_maps, core_ids=list(range(ncores)),
                               trace=trace)
    y = np.concatenate([res.results[c]["y"] for c in range(ncores)], axis=0)
    return y[:N].astype(np.float32), res


def kernel(input, adj_rows, adj_cols, adj_vals, W):
    y, _ = run(np.asarray(input), np.asarray(adj_rows), np.asarray(adj_cols),
               np.asarray(adj_vals), np.asarray(W), ncores=8)
    return y

